# revision 19
# baseline (speedup 1.0000x reference)
"""Trainium2 Bass kernel for the BAHDANAU+ group-recommendation model.

kernel(**inputs) takes the complete (unsharded) numpy inputs, distributes the
131072-query batch over 8 NeuronCores, runs the Bass kernel SPMD, and returns
the full [B, 1] float32 output.

Architecture (v3):
  Host-side TABLE transforms (query-independent):
    group_tab[g] = user_emb[members[g]] flattened to 96 cols, zero-padded to
    128 cols, bf16  ([500K, 128]).  item_tab = item_emb||genres, bf16,
    [100K, 32].  (Denormalizing the member->user lookup is a table transform,
    same category as the item_emb||genres concat.)

  Sharding: queries are assigned to cores by item-id range (12500 items per
  core) so each core's item ids fit int16 after rebasing; each core receives
  its own 12500-row slice of item_tab.  Within a core, queries are ordered by
  group-table chunk (16 chunks of 32768 rows, int16-indexable), each chunk
  padded to a static NPC positions with dummy (idx 0) queries.

  Gathers use the SWDGE dma_gather ucode (InstDMAGatherAnt): 16 group-chunk
  gathers (NPC indices each, 256B rows) + 4 item gathers (NPOS/4 indices,
  64B rows) per core -- ~20 Pool-engine instructions instead of the 640
  one-index-per-partition indirect DMAs of the baseline (~1.4us each).

  Compute per chunk of TPC tiles (tile = 128 positions):
    merge item cols into gbig; PE-transpose gbig -> gbigT; attention logits
    via matmul (lhsT=gbigT, rhs=attn_W); g = sum_k at_k*mem_k on DVE
    (row-major); PE-transpose g; assemble newGT=[gitT;gT;itT] on 96
    partitions; h = relu(new@W1+b1) via one matmul per tile; y = h@W2 on
    DVE; sigmoid(+b2) on the scalar engine.  All embedding math in bf16.

  Output y is produced in permuted position order; the host scatters it back
  with the inverse permutation.
"""

import sys

sys.path.insert(0, "/opt/trn_rl_repo")

from contextlib import ExitStack

import numpy as np
import ml_dtypes

import concourse.bacc as bacc
import concourse.bass as bass
import concourse.tile as tile
from concourse import library_config, masks, mybir
from concourse.bass_utils import run_bass_kernel_spmd

N_CORES = 8
P = 128
EMB = 32
B = 131_072
NUM_USERS = 1_000_000
NUM_ITEMS = 100_000
NUM_GROUPS = 500_000
N_CHUNKS = 16
CHUNK = 32_768          # group-table rows per chunk (int16 range)
ITEMS_PER_CORE = NUM_ITEMS // N_CORES
N_ITEM_GATHERS = 4
# 32 = gather bare 64B item rows (needs the %256 elem assert relaxed);
# 128 = gather 256B zero-padded rows and merge with an add (always safe).
ITEM_ELEM = 128
# Max indices per dma_gather instruction (HW-validated bound; must be a
# multiple of 128).  Larger requested gathers are split into pieces.
MAX_GATHER = 512

F32 = mybir.dt.float32
BF16 = mybir.dt.bfloat16
I16 = mybir.dt.int16
I32 = mybir.dt.int32
MULT = mybir.AluOpType.mult
ADD = mybir.AluOpType.add
AXX = mybir.AxisListType.X


def split_gather(nc, out_tile, tile0, in_ap, idx_tile, pos0, n, elem):
    """Emit dma_gather(s) for `n` positions starting at global position
    `pos0`, splitting into MAX_GATHER-index pieces.  out_tile is the
    [128, nt, elem] dst tile; tile0 = first dst tile-column; idx_tile is
    the [128, npos//16] wrapped index tile."""
    done = 0
    while done < n:
        piece = min(MAX_GATHER, n - done)
        assert piece % P == 0
        p0 = pos0 + done
        nc.gpsimd.dma_gather(
            out_ap=out_tile[:, tile0 + done // P : tile0 + (done + piece) // P, :],
            in_ap=in_ap,
            idxs_ap=idx_tile[:, p0 // 16 : (p0 + piece) // 16],
            num_idxs=piece,
            num_idxs_reg=piece,
            elem_size=elem,
        )
        done += piece


def build(npc):
    """Per-core Bass program for 16 chunks x npc positions (npc % 128 == 0)."""
    assert npc % P == 0
    tpc = npc // P               # tiles per chunk
    nt = N_CHUNKS * tpc          # total tiles
    npos = nt * P

    nc = bacc.Bacc(
        "TRN2",
        target_bir_lowering=False,
        debug=False,
        enable_asserts=False,
    )

    gidx = nc.dram_tensor("gidx", [P, npos // 16], I16, kind="ExternalInput")
    iidx = nc.dram_tensor("iidx", [P, npos // 16], I16, kind="ExternalInput")
    group_tab = nc.dram_tensor("group_tab", [NUM_GROUPS, P], BF16, kind="ExternalInput")
    item_tab = nc.dram_tensor(
        "item_tab", [ITEMS_PER_CORE, ITEM_ELEM], BF16, kind="ExternalInput"
    )
    attn_w_d = nc.dram_tensor("attn_w", [P, 3], BF16, kind="ExternalInput")
    w1_d = nc.dram_tensor("w1", [3 * EMB, 8], BF16, kind="ExternalInput")
    attnb_d = nc.dram_tensor("attnb", [P, 3], F32, kind="ExternalInput")
    b1_d = nc.dram_tensor("b1", [P, 8], F32, kind="ExternalInput")
    w2_d = nc.dram_tensor("w2", [P, 8], F32, kind="ExternalInput")
    b2_d = nc.dram_tensor("b2", [P, 1], F32, kind="ExternalInput")
    y_out = nc.dram_tensor("y_out", [P, nt], F32, kind="ExternalOutput")

    with tile.TileContext(nc) as tc, ExitStack() as ctx:
        singles = ctx.enter_context(tc.tile_pool(name="singles", bufs=1))
        gbigT_p = ctx.enter_context(tc.tile_pool(name="gbigT", bufs=2))
        small_p = ctx.enter_context(tc.tile_pool(name="small", bufs=2))
        tp_ps = ctx.enter_context(
            tc.tile_pool(name="tp_ps", bufs=2, space=bass.MemorySpace.PSUM)
        )
        gt_ps_p = ctx.enter_context(
            tc.tile_pool(name="gt_ps", bufs=2, space=bass.MemorySpace.PSUM)
        )
        at_ps_p = ctx.enter_context(
            tc.tile_pool(name="at_ps", bufs=2, space=bass.MemorySpace.PSUM)
        )
        h_ps_p = ctx.enter_context(
            tc.tile_pool(name="h_ps", bufs=2, space=bass.MemorySpace.PSUM)
        )

        # --- constants -------------------------------------------------
        ident = singles.tile([P, P], BF16)
        masks.make_identity(nc, ident[:])
        nc.gpsimd.load_library(library_config.mlp)
        attn_w_s = singles.tile([P, 3], BF16)
        nc.sync.dma_start(out=attn_w_s[:], in_=attn_w_d.ap())
        w1_s = singles.tile([3 * EMB, 8], BF16)
        nc.sync.dma_start(out=w1_s[:], in_=w1_d.ap())
        attnb_s = singles.tile([P, 3], F32)
        nc.sync.dma_start(out=attnb_s[:], in_=attnb_d.ap())
        b1_s = singles.tile([P, 8], F32)
        nc.sync.dma_start(out=b1_s[:], in_=b1_d.ap())
        w2_s = singles.tile([P, 8], F32)
        nc.sync.dma_start(out=w2_s[:], in_=w2_d.ap())
        b2_s = singles.tile([P, 1], F32)
        nc.sync.dma_start(out=b2_s[:], in_=b2_d.ap())

        gidx_s = singles.tile([P, npos // 16], I16)
        nc.sync.dma_start(out=gidx_s[:], in_=gidx.ap())
        iidx_s = singles.tile([P, npos // 16], I16)
        nc.sync.dma_start(out=iidx_s[:], in_=iidx.ap())

        gdst = singles.tile([P, nt, P], BF16)       # group rows (+item merged)
        idst = singles.tile([P, nt, ITEM_ELEM], BF16)  # item rows
        ypre = singles.tile([P, nt], F32)

        # --- item gathers ---------------------------------------------
        split_gather(nc, idst, 0, item_tab.ap(), iidx_s, 0, npos, ITEM_ELEM)

        for k in range(N_CHUNKS):
            ksl = slice(k * tpc, (k + 1) * tpc)
            # --- group-chunk gather (256B rows) ------------------------
            lo = k * CHUNK
            hi = min((k + 1) * CHUNK, NUM_GROUPS)
            split_gather(nc, gdst, k * tpc, group_tab.ap()[lo:hi, :],
                         gidx_s, k * npc, npc, P)
            # --- merge item cols into gbig -----------------------------
            gbig = gdst[:, ksl, :]
            if ITEM_ELEM == EMB:
                nc.vector.tensor_copy(out=gbig[:, :, 3 * EMB : 4 * EMB],
                                      in_=idst[:, ksl, :])
            else:
                # padded item rows: item data in cols 96:128, zeros elsewhere
                nc.vector.tensor_tensor(out=gbig[:], in0=gbig[:],
                                        in1=idst[:, ksl, :], op=ADD)

            # --- transpose tiles: gbig -> gbigT ------------------------
            gbigT = gbigT_p.tile([P, tpc, P], BF16, tag="gbigT")
            for g0 in range(0, tpc, 4):
                gsz = min(4, tpc - g0)
                pst = tp_ps.tile([P, 4, P], BF16, tag="tp")
                for j in range(gsz):
                    nc.tensor.matmul(
                        pst[:, j, :], lhsT=gbig[:, g0 + j, :], rhs=ident[:],
                        is_transpose=True, start=True, stop=True,
                        skip_group_check=True,
                    )
                nc.vector.tensor_copy(
                    out=gbigT[:, g0 : g0 + gsz, :], in_=pst[:, 0:gsz, :]
                )

            # --- attention logits: at = gi @ attn_W + b ----------------
            at_ps = at_ps_p.tile([P, tpc, 3], F32, tag="at")
            for j in range(tpc):
                nc.tensor.matmul(
                    at_ps[:, j, :], lhsT=gbigT[:, j, :], rhs=attn_w_s[:],
                    start=True, stop=True, skip_group_check=True,
                )
            at_sb = small_p.tile([P, tpc, 3], BF16, tag="at_sb")
            nc.vector.tensor_tensor(
                out=at_sb[:], in0=at_ps[:],
                in1=attnb_s[:].unsqueeze(1).to_broadcast([P, tpc, 3]), op=ADD,
            )

            # --- g = sum_k at_k * mem_k (row-major, DVE) ---------------
            g_sb = small_p.tile([P, tpc, EMB], BF16, tag="g_sb")
            tmp0 = small_p.tile([P, tpc, EMB], BF16, tag="tmp0")
            tmp1 = small_p.tile([P, tpc, EMB], BF16, tag="tmp1")
            nc.vector.tensor_tensor(
                out=tmp0[:], in0=gbig[:, :, 0:EMB],
                in1=at_sb[:, :, 0].unsqueeze(2).to_broadcast([P, tpc, EMB]),
                op=MULT,
            )
            nc.vector.tensor_tensor(
                out=tmp1[:], in0=gbig[:, :, EMB : 2 * EMB],
                in1=at_sb[:, :, 1].unsqueeze(2).to_broadcast([P, tpc, EMB]),
                op=MULT,
            )
            nc.vector.tensor_tensor(out=tmp0[:], in0=tmp0[:], in1=tmp1[:], op=ADD)
            nc.vector.tensor_tensor(
                out=tmp1[:], in0=gbig[:, :, 2 * EMB : 3 * EMB],
                in1=at_sb[:, :, 2].unsqueeze(2).to_broadcast([P, tpc, EMB]),
                op=MULT,
            )
            nc.vector.tensor_tensor(out=g_sb[:], in0=tmp0[:], in1=tmp1[:], op=ADD)

            # --- newGT = [gitT; gT; itT] on 96 partitions --------------
            newGT = gbigT_p.tile([3 * EMB, tpc, P], BF16, tag="newGT")
            for g0 in range(0, tpc, 4):
                gsz = min(4, tpc - g0)
                sl = slice(g0, g0 + gsz)
                gt_ps = gt_ps_p.tile([EMB, 4, P], BF16, tag="gt")
                for j in range(gsz):
                    nc.tensor.matmul(
                        gt_ps[:, j, :], lhsT=g_sb[:, g0 + j, :], rhs=ident[:],
                        is_transpose=True, start=True, stop=True,
                        skip_group_check=True,
                    )
                # gT -> partitions 32:64
                nc.scalar.copy(out=newGT[EMB : 2 * EMB, sl, :],
                               in_=gt_ps[:, 0:gsz, :])
                # gitT = gT * itT -> partitions 0:32
                nc.vector.tensor_tensor(
                    out=newGT[0:EMB, sl, :], in0=gt_ps[:, 0:gsz, :],
                    in1=gbigT[3 * EMB : 4 * EMB, sl, :], op=MULT,
                )
                # itT -> partitions 64:96
                nc.scalar.copy(
                    out=newGT[2 * EMB : 3 * EMB, sl, :],
                    in_=gbigT[3 * EMB : 4 * EMB, sl, :],
                )

            # --- h = relu(new @ W1 + b1) -------------------------------
            h_ps = h_ps_p.tile([P, tpc, 8], F32, tag="h")
            for j in range(tpc):
                nc.tensor.matmul(
                    h_ps[:, j, :], lhsT=newGT[:, j, :], rhs=w1_s[:],
                    start=True, stop=True, skip_group_check=True,
                )
            h_sb = small_p.tile([P, tpc, 8], F32, tag="h_sb")
            nc.vector.tensor_tensor(
                out=h_sb[:], in0=h_ps[:],
                in1=b1_s[:].unsqueeze(1).to_broadcast([P, tpc, 8]), op=ADD,
            )
            nc.vector.tensor_scalar_max(h_sb[:], h_sb[:], 0.0)

            # --- y = h @ W2 --------------------------------------------
            hw = small_p.tile([P, tpc, 8], F32, tag="hw")
            nc.vector.tensor_tensor(
                out=hw[:], in0=h_sb[:],
                in1=w2_s[:].unsqueeze(1).to_broadcast([P, tpc, 8]), op=MULT,
            )
            nc.vector.tensor_reduce(
                out=ypre[:, ksl], in_=hw[:], axis=AXX, op=ADD
            )

        # --- sigmoid(y + b2) and store --------------------------------
        ysig = singles.tile([P, nt], F32)
        nc.scalar.activation(
            out=ysig[:], in_=ypre[:],
            func=mybir.ActivationFunctionType.Sigmoid,
            bias=b2_s[:, 0:1], scale=1.0,
        )
        nc.sync.dma_start(out=y_out.ap(), in_=ysig[:])

    nc.compile()
    return nc


def wrap_idx(vals, npos):
    """[n] -> [128, npos//16] int16: position j -> partition j%16 (replicated
    across the 8 16-partition groups), column j//16."""
    full = np.zeros(npos, np.int16)
    full[: len(vals)] = vals
    block = full.reshape(npos // 16, 16).T
    return np.ascontiguousarray(np.tile(block, (8, 1)))


def prep_host_inputs(inputs, n_cores=N_CORES):
    """Tables (bf16), per-core chunk-sorted index layouts, output scatter map."""
    grp = np.asarray(inputs["group_inputs"]).astype(np.int64).reshape(-1)
    itm = np.asarray(inputs["item_inputs"]).astype(np.int64).reshape(-1)
    nq = grp.shape[0]

    user_emb = np.asarray(inputs["user_emb"], np.float32)
    members = np.asarray(inputs["members"]).astype(np.int64)
    group_tab = np.zeros((members.shape[0], P), ml_dtypes.bfloat16)
    group_tab[:, : 3 * EMB] = (
        user_emb[members.reshape(-1)]
        .reshape(members.shape[0], 3 * EMB)
        .astype(ml_dtypes.bfloat16)
    )
    item_rows = np.concatenate(
        [
            np.asarray(inputs["item_emb"], np.float32),
            np.asarray(inputs["genres"], np.float32),
        ],
        axis=1,
    ).astype(ml_dtypes.bfloat16)
    if ITEM_ELEM == EMB:
        item_tab = np.ascontiguousarray(item_rows)
    else:
        item_tab = np.zeros((NUM_ITEMS, ITEM_ELEM), ml_dtypes.bfloat16)
        item_tab[:, 3 * EMB : 4 * EMB] = item_rows

    # --- assign queries to cores by item range, sort by group chunk ----
    core_of = itm // ITEMS_PER_CORE
    per_core = []  # (perm_chunks: list of global query idx arrays per chunk)
    max_nk = 1
    for c in range(n_cores):
        qc = np.nonzero(core_of == c)[0]
        chunk = grp[qc] // CHUNK
        chunks = [qc[chunk == k] for k in range(N_CHUNKS)]
        per_core.append(chunks)
        if len(qc):
            max_nk = max(max_nk, max(len(x) for x in chunks))
    npc = -(-max_nk // P) * P  # round up to 128
    npos = N_CHUNKS * npc

    in_extra = []
    perms = []
    for c in range(n_cores):
        gl = np.zeros(npos, np.int16)
        il = np.zeros(npos, np.int16)
        pm = np.full(npos, -1, np.int64)
        for k, qk in enumerate(per_core[c]):
            o = k * npc
            n = len(qk)
            gl[o : o + n] = (grp[qk] - k * CHUNK).astype(np.int16)
            il[o : o + n] = (itm[qk] - c * ITEMS_PER_CORE).astype(np.int16)
            pm[o : o + n] = qk
        in_extra.append(
            {
                "gidx": wrap_idx(gl, npos),
                "iidx": wrap_idx(il, npos),
                "item_tab": np.ascontiguousarray(
                    item_tab[c * ITEMS_PER_CORE : (c + 1) * ITEMS_PER_CORE]
                ),
            }
        )
        perms.append(pm)

    attn_W = np.asarray(inputs["attn_W"], np.float32)
    attn_b = np.asarray(inputs["attn_b"], np.float32)
    w1 = np.asarray(inputs["pred_W1"], np.float32)
    b1 = np.asarray(inputs["pred_b1"], np.float32)
    w2 = np.asarray(inputs["pred_W2"], np.float32)
    b2 = np.asarray(inputs["pred_b2"], np.float32)
    weights = {
        "attn_w": np.ascontiguousarray(attn_W.astype(ml_dtypes.bfloat16)),
        "w1": np.ascontiguousarray(w1.astype(ml_dtypes.bfloat16)),
        "attnb": np.ascontiguousarray(np.tile(attn_b[None, :], (P, 1))),
        "b1": np.ascontiguousarray(np.tile(b1[None, :], (P, 1))),
        "w2": np.ascontiguousarray(np.tile(w2[:, 0][None, :], (P, 1))),
        "b2": np.ascontiguousarray(np.tile(b2.reshape(1, 1), (P, 1))),
    }
    return group_tab, weights, in_extra, perms, npc, nq


def make_in_maps(group_tab, weights, in_extra):
    return [{"group_tab": group_tab, **weights, **ex} for ex in in_extra]


_NC_CACHE = {}


def kernel(**inputs) -> np.ndarray:
    group_tab, weights, in_extra, perms, npc, nq = prep_host_inputs(inputs)
    if npc not in _NC_CACHE:
        _NC_CACHE[npc] = build(npc)
    nc = _NC_CACHE[npc]
    in_maps = make_in_maps(group_tab, weights, in_extra)
    res = run_bass_kernel_spmd(nc, in_maps, core_ids=list(range(N_CORES)))
    y = np.zeros(nq, np.float32)
    for c in range(N_CORES):
        yc = res.results[c]["y_out"]          # [128, nt]; position j -> [j%128, j//128]
        flat = np.ascontiguousarray(yc.T).reshape(-1)
        pm = perms[c]
        valid = pm >= 0
        y[pm[valid]] = flat[valid]
    return y.reshape(-1, 1).astype(np.float32)
